# revision 6
# baseline (speedup 1.0000x reference)
"""Dynamic GQA attention (nn_DGQA) on 8 Trainium2 NeuronCores.

Strategy (v2)
-------------
The dynamic query-head -> kv-head assignment is computed on the host and the
device program is SPECIALIZED to the resulting per-core local grouping
pattern (cached per pattern; identical across cores for the uniform case,
with an identity-map fallback that reproduces the v1 per-head-duplicated
layout).  With g distinct kv heads per core the k/v projections are computed
at width g*64 instead of 8*64, and the kT rows needed by the row-packed QK
pairs are fanned out by cheap DVE copies.

Sharding: core c handles batch b = c//2 and half of the query heads
(half = c%2).  Each core computes a partial output projection; the host sums
the two partials per batch and adds the bias.

Device kernel (per core, all matmuls bf16, fp32 accumulation):
  xt [1024,2048] = x[b].T
  qT = wq.T @ xt            (wq pre-scaled by D^-0.5 on host)
  ktd = wk.T @ xt           (dedup'd: g kv heads)  -> DVE fan-out to kt_qk
  v  = xt.T @ wv            (dedup'd, strided into "vplus" with ones col)
  per head pair (row-tiled QK at K=64, 2 heads concurrent):
    scoresT[key, qp] in PSUM -> exp on ScalarE -> bf16 SBUF
    PV: outT[d, qp] (+ sums row via ones column), v stationary
  softmax normalization: sums rows staged at partitions {0,32,64,96} of a
  [128, 2*QW] tile; ONE ln + ONE exp per t-tile covers all 4 pairs
  (ScalarE time is free-dim-bound, so batching partitions is ~4x cheaper);
  reciprocals broadcast with col-tiled k=1 matmuls, multiplied on VectorE
  proj: out[row, :] partial = outT.T @ wp
"""

import numpy as np
import ml_dtypes

B, P, DIM, H, HKV = 4, 2048, 1024, 16, 8
D = DIM // H          # 64
NCORES = 8
HPC = H // 2          # query heads per core = 8
DPC = HPC * D         # query head-dims per core = 512
VW = D + 1            # v columns per head incl. ones column = 65

BF16 = ml_dtypes.bfloat16


# ----------------------------------------------------------------- host math

def _ratios_np(k_bhpd: np.ndarray, cache: np.ndarray) -> np.ndarray:
    """Numpy replica of the reference's _ratios (fp32, round-half-even)."""
    mags = np.sqrt((k_bhpd * k_bhpd).sum(axis=(2, 3))).sum(axis=0)
    diff = np.abs(cache - mags)
    r = np.round(diff / diff.sum() * H).astype(np.int64)
    while r.sum() > H:
        r[np.argmax(r)] -= 1
    while r.sum() < H:
        r[np.argmin(r)] += 1
    return r


def _kv_id(x: np.ndarray, Wk: np.ndarray, cache: np.ndarray) -> np.ndarray:
    k = (x.reshape(B * P, DIM).astype(np.float32) @ Wk.astype(np.float32))
    k = k.reshape(B, P, HKV, D).transpose(0, 2, 1, 3)
    r = _ratios_np(k, cache.astype(np.float32))
    return np.searchsorted(np.cumsum(r), np.arange(H), side="right")


# ----------------------------------------------------- walrus wait splitting

def _split_wide_waits(nc, max_waits=1):
    """This toolchain's walrus allows only one sync-wait per instruction;
    move extra waits onto preceding NOPs on the same engine."""
    import bass_rust
    import concourse.mybir as mybir

    n = 0
    for f in nc.m.functions:
        for blk in f.blocks:
            out = []
            changed = False
            for ins in blk.instructions:
                si = ins.sync_info
                if si is not None and si.on_wait is not None and \
                        len(si.on_wait) > max_waits:
                    waits = list(si.on_wait)
                    keep = waits[-max_waits:]
                    extra = waits[:-max_waits]
                    for j in range(0, len(extra), max_waits):
                        n += 1
                        nop = mybir.InstNoOp(
                            name=f"waitsplit-{n}", ins=[], outs=[])
                        nop.engine = ins.engine
                        nop.sync_info = bass_rust.SyncInfo(
                            on_wait=extra[j:j + max_waits], on_update=[])
                        out.append(nop)
                    ins.sync_info = bass_rust.SyncInfo(
                        on_wait=keep, on_update=list(si.on_update or []))
                    changed = True
                out.append(ins)
            if changed:
                blk.instructions = out
    return n


# ------------------------------------------------------------ device program

def build_program(lmap, p_len: int = P, split_waits: bool = True):
    """Build the SPMD Bass/Tile program, specialized to the per-core local
    query-head -> kv-slot map `lmap` (tuple of HPC ints in [0, g))."""
    from contextlib import ExitStack

    import concourse.bass as bass
    import concourse.tile as tile
    from concourse import mybir

    F32 = mybir.dt.float32
    BF = mybir.dt.bfloat16
    EXP = mybir.ActivationFunctionType.Exp
    LN = mybir.ActivationFunctionType.Ln

    g = max(lmap) + 1         # distinct kv heads on this core
    GW = g * D                # k/v projection width
    NKT = (g + 1) // 2        # dedup'd kT row-tiles of 128

    NKC = p_len // 128            # key chunks
    NQT = max(p_len // 512, 1)    # q tiles
    QW = min(512, p_len)          # q tile width
    NRM = p_len // 128            # x row chunks
    RPT = NRM // NQT              # row chunks per q tile
    NDIN = DIM // 128             # contraction chunks for projections
    NPAIR = HPC // 2
    LAG = 4                       # PV chunks behind QK

    nc = bass.Bass("TRN2", target_bir_lowering=False, debug=False,
                   num_devices=NCORES)
    xt_d = nc.dram_tensor("xt", [DIM, p_len], BF, kind="ExternalInput").ap()
    wq_d = nc.dram_tensor("wq", [DIM, DPC], BF, kind="ExternalInput").ap()
    wk_d = nc.dram_tensor("wk", [DIM, GW], BF, kind="ExternalInput").ap()
    wv_d = nc.dram_tensor("wv", [DIM, GW], BF, kind="ExternalInput").ap()
    wp_d = nc.dram_tensor("wp", [DPC, DIM], BF, kind="ExternalInput").ap()
    out_d = nc.dram_tensor("out", [p_len, DIM], F32, kind="ExternalOutput").ap()

    with tile.TileContext(nc) as tc, ExitStack() as ctx:
        sbw = ctx.enter_context(tc.tile_pool(name="sbw", bufs=1))
        sbx = ctx.enter_context(tc.tile_pool(name="sbx", bufs=1))
        sbqk = ctx.enter_context(tc.tile_pool(name="sbqk", bufs=1))
        sbeg = ctx.enter_context(tc.tile_pool(name="sbeg", bufs=7))
        sbot = ctx.enter_context(tc.tile_pool(name="sbot", bufs=2))
        sbo = ctx.enter_context(tc.tile_pool(name="sbo", bufs=3))
        sbr = ctx.enter_context(tc.tile_pool(name="sbr", bufs=9))
        sbn = ctx.enter_context(tc.tile_pool(name="sbn", bufs=1))
        psb = ctx.enter_context(tc.tile_pool(name="psb", bufs=2, space="PSUM"))
        pssg = ctx.enter_context(tc.tile_pool(name="pssg", bufs=2, space="PSUM"))
        pspv = ctx.enter_context(tc.tile_pool(name="pspv", bufs=2, space="PSUM"))

        # ---- input loads -------------------------------------------------
        wq_sb = [sbw.tile([128, DPC], BF, tag=f"wq{i}", name=f"wq{i}") for i in range(NDIN)]
        wk_sb = [sbw.tile([128, GW], BF, tag=f"wk{i}", name=f"wk{i}") for i in range(NDIN)]
        wv_sb = [sbw.tile([128, GW], BF, tag=f"wv{i}", name=f"wv{i}") for i in range(NDIN)]
        xt_sb = [sbx.tile([128, p_len], BF, tag=f"xt{i}", name=f"xt{i}") for i in range(NDIN)]
        wp_sb = [sbw.tile([128, DIM], BF, tag=f"wp{i}", name=f"wp{i}") for i in range(DPC // 128)]
        for i in range(NDIN):
            nc.sync.dma_start(xt_sb[i][:], xt_d[128 * i:128 * (i + 1), :])
            nc.sync.dma_start(wk_sb[i][:], wk_d[128 * i:128 * (i + 1), :])
        for i in range(NDIN):
            nc.sync.dma_start(wq_sb[i][:], wq_d[128 * i:128 * (i + 1), :])
            nc.sync.dma_start(wv_sb[i][:], wv_d[128 * i:128 * (i + 1), :])
        for i in range(DPC // 128):
            nc.sync.dma_start(wp_sb[i][:], wp_d[128 * i:128 * (i + 1), :])

        # ones rows at partitions {0,32,64,96} for the k=1 broadcast matmuls
        ones_sb = sbn.tile([128, 64], BF, tag="ones", name="ones")
        nc.vector.memset(ones_sb[:], 1.0)

        # normalization staging: sums rows / reciprocals for one t-tile live
        # at partitions {32p}; ScalarE cost is free-dim-bound, so one ln+exp
        # pair covers all 4 pairs of a t.
        sm_st = sbn.tile([128, 2 * QW], F32, tag="sm", name="sm")
        lg_st = sbn.tile([128, 2 * QW], F32, tag="lg", name="lg")
        rc_st = sbn.tile([128, 2 * QW], BF, tag="rc", name="rc")
        nc.vector.memset(sm_st[:], 1.0)

        # Touch Ln and Exp immediately so the ACT table set loads at kernel
        # start, long before the first real exp.
        warm = sbn.tile([1, 8], F32, tag="warm", name="warm")
        nc.vector.memset(warm[:], 1.0)
        nc.scalar.activation(warm[:], warm[:], LN)
        nc.scalar.activation(warm[:], warm[:], EXP)

        # ---- stage B: projection chain emitters ---------------------------
        # qT: [DPC, p_len]; ktd: dedup'd [g*64, p_len]; kt_qk: per-pair
        # [128, p_len] with the pair's two kv-head row blocks at partitions
        # 0:64 / 64:128 (so QK pairs row-pack in the PE array).
        qt_sb = [sbqk.tile([128, p_len], BF, tag=f"qt{m}", name=f"qt{m}") for m in range(DPC // 128)]
        ktd_sb = [sbqk.tile([128, p_len], BF, tag=f"ktd{m}", name=f"ktd{m}") for m in range(NKT)]
        ktq_sb = [sbqk.tile([128, p_len], BF, tag=f"ktq{m}", name=f"ktq{m}") for m in range(NPAIR)]
        vplus_sb = sbqk.tile([128, NKC * g * VW], BF, tag="vplus", name="vplus")
        vp3 = vplus_sb[:].rearrange("p (kc g w) -> p kc g w", kc=NKC, g=g)

        def vplus(kc, j):
            off = (kc * g + j) * VW
            return vplus_sb[:, off:off + VW]

        nc.vector.memset(vp3[:, :, :, D:VW], 1.0)

        def kt_dup(jt, t):
            """Fan dedup'd kT tile jt's fresh t-slice out to the QK tiles."""
            for pr in range(NPAIR):
                for half in (0, 1):
                    kv = lmap[2 * pr + half]
                    if kv // 2 == jt:
                        src = ktd_sb[jt][64 * (kv % 2):64 * (kv % 2) + 64,
                                         QW * t:QW * (t + 1)]
                        nc.vector.tensor_copy(
                            ktq_sb[pr][64 * half:64 * half + 64,
                                       QW * t:QW * (t + 1)], src)

        def qk_chain_gen(dst, w_sb, m, t, cols, post=None):
            ps = psb.tile([128, 512], F32, tag="psb", name="psb")
            for kd in range(NDIN):
                nc.tensor.matmul(
                    ps[0:cols, 0:QW], w_sb[kd][:, 128 * m:128 * m + cols],
                    xt_sb[kd][:, QW * t:QW * (t + 1)],
                    start=(kd == 0), stop=(kd == NDIN - 1))
                if kd % 2 == 1 and kd < NDIN - 1:
                    yield
            nc.vector.tensor_copy(dst[m][0:cols, QW * t:QW * (t + 1)],
                                  ps[0:cols, 0:QW])
            if post is not None:
                post()

        def v_chain_gen(rm):
            ps = psb.tile([128, 512], F32, tag="psb", name="psb")
            for kd in range(NDIN):
                nc.tensor.matmul(
                    ps[:, 0:GW], xt_sb[kd][:, 128 * rm:128 * (rm + 1)],
                    wv_sb[kd][:],
                    start=(kd == 0), stop=(kd == NDIN - 1))
                if kd % 2 == 1 and kd < NDIN - 1:
                    yield
            nc.vector.tensor_copy(
                vp3[:, rm, :, 0:D],
                ps[:, 0:GW].rearrange("p (g d) -> p g d", g=g))

        def proj_gen(t, rj, ot_tiles):
            o_sb = sbo.tile([128, DIM], F32, tag="osb", name="osb")
            for e2 in range(DIM // 512):
                ps = psb.tile([128, 512], F32, tag="psb", name="psb")
                for pair in range(NPAIR):
                    nc.tensor.matmul(
                        ps[:],
                        ot_tiles[pair][:, 128 * rj:128 * (rj + 1)],
                        wp_sb[pair][:, 512 * e2:512 * (e2 + 1)],
                        start=(pair == 0), stop=(pair == NPAIR - 1))
                    if pair == 1:
                        yield
                nc.vector.tensor_copy(o_sb[:, 512 * e2:512 * (e2 + 1)], ps[:])
                yield
            row0 = (t * RPT + rj) * 128
            nc.sync.dma_start(out_d[row0:row0 + 128, :], o_sb[:])

        import heapq

        total_chunks = NQT * NPAIR * NKC
        pump_q = []   # (deadline, seq, earliest, gen)
        pump_seq = [0]

        def add_gen(deadline, earliest, gen):
            pump_seq[0] += 1
            heapq.heappush(pump_q, (deadline, pump_seq[0], earliest, gen))

        # first QK tile of qt and the kt tiles it needs, eagerly (not pumped)
        def run_gen(gen):
            for _ in gen:
                pass

        run_gen(qk_chain_gen(qt_sb, wq_sb, 0, 0, 128))
        for jt in range(NKT):
            run_gen(qk_chain_gen(ktd_sb, wk_sb, jt,
                                 0, min(128, GW - 128 * jt),
                                 post=(lambda jt=jt: kt_dup(jt, 0))))

        for rm in range(NKC):
            add_gen(max(rm + 1, 0), 0, v_chain_gen(rm))
        for t in range(NQT):
            for m in range(DPC // 128):
                if m == 0 and t == 0:
                    continue
                dl = max((NQT * t + m) * NKC - 2, 0)
                add_gen(dl, 0, qk_chain_gen(qt_sb, wq_sb, m, t, 128))
            for jt in range(NKT):
                if t == 0:
                    continue
                dl = max((NQT * t) * NKC - 4, 0)
                add_gen(dl, 0, qk_chain_gen(
                    ktd_sb, wk_sb, jt, t, min(128, GW - 128 * jt),
                    post=(lambda jt=jt, t=t: kt_dup(jt, t))))

        pump_state = {"gen": None, "dl": 0}

        def pump(gg, budget=1):
            steps = 0
            while True:
                if pump_state["gen"] is None:
                    if not pump_q or pump_q[0][2] > gg:
                        return
                    dl, _, _, gen = heapq.heappop(pump_q)
                    pump_state["gen"] = gen
                    pump_state["dl"] = dl
                urgent = pump_state["dl"] <= gg + 2
                if steps >= budget and not urgent:
                    return
                try:
                    next(pump_state["gen"])
                    steps += 1
                except StopIteration:
                    pump_state["gen"] = None

        # ---- stage C + D: attention, normalization, projection ------------
        units = [(t, pair) for t in range(NQT) for pair in range(NPAIR)]

        class Unit:
            pass

        def start_unit(i):
            u = Unit()
            u.t, u.pair = units[i]
            u.h0, u.h1 = 2 * u.pair, 2 * u.pair + 1
            u.qt, u.kt = qt_sb[u.pair], ktq_sb[u.pair]
            u.pv0 = pspv.tile([128, QW], F32, tag="pv", name="pv")
            u.pv1 = pspv.tile([128, QW], F32, tag="pv", name="pv")
            u.egs = [None] * NKC
            u.fa_done = u.rc_done = False
            u.fb_done = 0
            return u

        def qk_exp(u, kc):
            sg = pssg.tile([128, 2 * QW], F32, tag="sg", name="sg")
            nc.tensor.matmul(
                sg[:, 0:QW], u.kt[0:64, 128 * kc:128 * (kc + 1)],
                u.qt[0:64, QW * u.t:QW * (u.t + 1)], start=True, stop=True)
            nc.tensor.matmul(
                sg[:, QW:2 * QW], u.kt[64:128, 128 * kc:128 * (kc + 1)],
                u.qt[64:128, QW * u.t:QW * (u.t + 1)], start=True, stop=True)
            eg = sbeg.tile([128, 2 * QW], BF, tag="eg", name="eg")
            nc.scalar.activation(eg[:], sg[:], EXP)
            u.egs[kc] = eg

        def pv_mm(u, kc):
            nc.tensor.matmul(
                u.pv0[0:VW, :], vplus(kc, lmap[u.h0]), u.egs[kc][:, 0:QW],
                start=(kc == 0), stop=(kc == NKC - 1))
            nc.tensor.matmul(
                u.pv1[0:VW, :], vplus(kc, lmap[u.h1]), u.egs[kc][:, QW:2 * QW],
                start=(kc == 0), stop=(kc == NKC - 1))
            u.egs[kc] = None

        def finalize_a(u):
            # copy unnormalized outT to SBUF (frees the pv psum banks) and
            # stage the raw sums rows at partition 32*pair of sm_st.
            u.s0 = sbr.tile([64, QW], F32, tag="s", name="s")
            u.s1 = sbr.tile([64, QW], F32, tag="s", name="s")
            nc.vector.tensor_copy(u.s0[:], u.pv0[0:D, :])
            nc.vector.tensor_copy(u.s1[:], u.pv1[0:D, :])
            pp = 32 * u.pair
            nc.vector.tensor_copy(sm_st[pp:pp + 1, 0:QW], u.pv0[D:VW, :])
            nc.vector.tensor_copy(sm_st[pp:pp + 1, QW:2 * QW], u.pv1[D:VW, :])
            u.fa_done = True

        def recip_t():
            # reciprocal of all 8 sums rows of this t as exp(-ln s):
            # both functions live in one ACT table set, and the batched
            # [128, 2QW] layout makes the two activations cost the same as
            # a single pair's would.
            nc.scalar.activation(lg_st[:], sm_st[:], LN)
            nc.scalar.activation(rc_st[:], lg_st[:], EXP, scale=-1.0)

        def finalize_b(u, ot_tiles):
            # broadcast reciprocals across partitions (k=1 matmuls, col-tiled
            # into one psum tile) and scale on VectorE.
            pp = 32 * u.pair
            rb = psb.tile([128, QW], F32, tag="psb", name="psb")
            nc.tensor.matmul(rb[0:64, :], ones_sb[pp:pp + 1, :],
                             rc_st[pp:pp + 1, 0:QW], start=True, stop=True,
                             tile_position=(pp, 0))
            nc.tensor.matmul(rb[64:128, :], ones_sb[pp:pp + 1, :],
                             rc_st[pp:pp + 1, QW:2 * QW], start=True,
                             stop=True, tile_position=(pp, 64))
            ot = sbot.tile([128, QW], BF, tag=f"ot{u.pair}", name=f"ot{u.pair}")
            nc.vector.tensor_mul(ot[0:64, :], u.s0[:], rb[0:64, :])
            nc.vector.tensor_mul(ot[64:128, :], u.s1[:], rb[64:128, :])
            ot_tiles[u.pair] = ot

        ot_by_t = {t: [None] * NPAIR for t in range(NQT)}

        def step_prev(u, kc, gg):
            # deferred post-processing of the previous unit(s), spread over
            # this unit's chunk stream
            if kc >= LAG and not u.fa_done:
                finalize_a(u)
            elif u.pair == NPAIR - 1 and u.fa_done:
                if kc >= LAG + 2 and not u.rc_done:
                    recip_t()
                    u.rc_done = True
                elif u.rc_done and u.fb_done < NPAIR and kc >= LAG + 3 + u.fb_done:
                    finalize_b(u.t_units[u.fb_done], ot_by_t[u.t])
                    u.fb_done += 1
                    if u.fb_done == NPAIR:
                        for rj in range(RPT):
                            add_gen(gg + 8 + rj, gg + 1,
                                    proj_gen(u.t, rj, ot_by_t[u.t]))

        def drain_prev(u, gg):
            if not u.fa_done:
                finalize_a(u)
            if u.pair == NPAIR - 1:
                if not u.rc_done:
                    recip_t()
                    u.rc_done = True
                while u.fb_done < NPAIR:
                    finalize_b(u.t_units[u.fb_done], ot_by_t[u.t])
                    u.fb_done += 1
                for rj in range(RPT):
                    add_gen(gg + 8 + rj, 0, proj_gen(u.t, rj, ot_by_t[u.t]))

        prev = None
        t_units = []
        cur = start_unit(0)
        for i in range(len(units)):
            t_units.append(cur)
            if cur.pair == NPAIR - 1:
                cur.t_units = t_units
                t_units = []
            for kc in range(NKC):
                gg = i * NKC + kc
                pump(gg)
                qk_exp(cur, kc)
                gk = kc - LAG
                if gk >= 0:
                    pv_mm(cur, gk)
                elif prev is not None:
                    pv_mm(prev, NKC + gk)
                if prev is not None:
                    step_prev(prev, kc, gg)
            if prev is not None and (not prev.fa_done or
                                     (prev.pair == NPAIR - 1 and
                                      prev.fb_done < NPAIR)):
                drain_prev(prev, i * NKC + NKC - 1)
            prev, cur = cur, (start_unit(i + 1) if i + 1 < len(units) else None)
        for gk in range(NKC - LAG, NKC):
            pv_mm(prev, gk)
            pump(total_chunks)
        drain_prev(prev, total_chunks)
        gg = total_chunks
        while pump_q or pump_state["gen"] is not None:
            pump(gg, budget=100)
            gg += 1

    if split_waits:
        _split_wide_waits(nc, max_waits=1)
    return nc


_PROGRAMS = {}


def _get_program(lmap):
    if lmap not in _PROGRAMS:
        _PROGRAMS[lmap] = build_program(lmap, P)
    return _PROGRAMS[lmap]


# ------------------------------------------------------------------- kernel

def make_in_maps(x, Wq, Wk, Wv, Wp, bp, cache):
    x = np.asarray(x, np.float32)
    Wq = np.asarray(Wq, np.float32)
    Wk = np.asarray(Wk, np.float32)
    Wv = np.asarray(Wv, np.float32)
    Wp = np.asarray(Wp, np.float32)
    kv_id = _kv_id(x, Wk, np.asarray(cache, np.float32))

    # per-core local maps; specialize if identical across cores
    maps, dists = [], []
    for c in range(NCORES):
        half = c % 2
        ids = [int(kv_id[h]) for h in range(half * HPC, (half + 1) * HPC)]
        uniq = sorted(set(ids))
        maps.append(tuple(uniq.index(i) for i in ids))
        dists.append(uniq)
    if all(m == maps[0] for m in maps):
        lmap = maps[0]
    else:
        lmap = tuple(range(HPC))
        dists = [[int(kv_id[h]) for h in range((c % 2) * HPC,
                                               (c % 2 + 1) * HPC)]
                 for c in range(NCORES)]

    scale = 1.0 / np.sqrt(D)
    in_maps = []
    xt_b = [np.ascontiguousarray(x[b].T).astype(BF16) for b in range(B)]
    for c in range(NCORES):
        b, half = divmod(c, 2)
        wk_c = np.concatenate(
            [Wk[:, d * D:(d + 1) * D] for d in dists[c]], axis=1)
        wv_c = np.concatenate(
            [Wv[:, d * D:(d + 1) * D] for d in dists[c]], axis=1)
        in_maps.append({
            "xt": xt_b[b],
            "wq": (Wq[:, half * DPC:(half + 1) * DPC] * scale).astype(BF16),
            "wk": np.ascontiguousarray(wk_c).astype(BF16),
            "wv": np.ascontiguousarray(wv_c).astype(BF16),
            "wp": np.ascontiguousarray(
                Wp[half * DPC:(half + 1) * DPC, :]).astype(BF16),
        })
    return in_maps, lmap


_WARMED = set()


def kernel(x, Wq, Wk, Wv, Wp, bp, cache, _trace=False):
    from concourse.bass_utils import run_bass_kernel_spmd

    in_maps, lmap = make_in_maps(x, Wq, Wk, Wv, Wp, bp, cache)
    nc = _get_program(lmap)
    if lmap not in _WARMED:
        # First execution on a cold NEFF has been observed racing the ACT
        # table load; run once and discard.
        run_bass_kernel_spmd(nc, in_maps, core_ids=list(range(NCORES)),
                             trace=False)
        _WARMED.add(lmap)
    res = run_bass_kernel_spmd(nc, in_maps, core_ids=list(range(NCORES)),
                               trace=_trace)
    bp32 = np.asarray(bp, np.float32)
    out = np.empty((B, P, DIM), np.float32)
    for b in range(B):
        out[b] = res.results[2 * b]["out"] + res.results[2 * b + 1]["out"] + bp32
    if _trace:
        kernel.last_exec_time_ns = res.exec_time_ns
    return out


# revision 9
# speedup vs baseline: 1.0028x; 1.0028x over previous
"""Dynamic GQA attention (nn_DGQA) on 8 Trainium2 NeuronCores.

Strategy (v2)
-------------
The dynamic query-head -> kv-head assignment is computed on the host and the
device program is SPECIALIZED to the resulting per-core local grouping
pattern (cached per pattern; identical across cores for the uniform case,
with an identity-map fallback that reproduces the v1 per-head-duplicated
layout).  With g distinct kv heads per core the k/v projections are computed
at width g*64 instead of 8*64, and the kT rows needed by the row-packed QK
pairs are fanned out by cheap DVE copies.

Sharding: core c handles batch b = c//2 and half of the query heads
(half = c%2).  Each core computes a partial output projection; the host sums
the two partials per batch and adds the bias.

Device kernel (per core, all matmuls bf16, fp32 accumulation):
  xt [1024,2048] = x[b].T
  qT = wq.T @ xt            (wq pre-scaled by D^-0.5 on host)
  ktd = wk.T @ xt           (dedup'd: g kv heads)  -> DVE fan-out to kt_qk
  v  = xt.T @ wv            (dedup'd, strided into "vplus" with ones col)
  per head pair (row-tiled QK at K=64, 2 heads concurrent):
    scoresT[key, qp] in PSUM -> exp on ScalarE -> bf16 SBUF
    PV: outT[d, qp] (+ sums row via ones column), v stationary
  softmax normalization: sums rows staged at partitions {0,32,64,96} of a
  [128, 2*QW] tile; ONE ln + ONE exp per t-tile covers all 4 pairs
  (ScalarE time is free-dim-bound, so batching partitions is ~4x cheaper);
  reciprocals broadcast with col-tiled k=1 matmuls, multiplied on VectorE
  proj: out[row, :] partial = outT.T @ wp
"""

import numpy as np
import ml_dtypes

B, P, DIM, H, HKV = 4, 2048, 1024, 16, 8
D = DIM // H          # 64
NCORES = 8
HPC = H // 2          # query heads per core = 8
DPC = HPC * D         # query head-dims per core = 512
VW = D + 1            # v columns per head incl. ones column = 65

BF16 = ml_dtypes.bfloat16


# ----------------------------------------------------------------- host math

def _ratios_np(k_bhpd: np.ndarray, cache: np.ndarray) -> np.ndarray:
    """Numpy replica of the reference's _ratios (fp32, round-half-even)."""
    mags = np.sqrt((k_bhpd * k_bhpd).sum(axis=(2, 3))).sum(axis=0)
    diff = np.abs(cache - mags)
    r = np.round(diff / diff.sum() * H).astype(np.int64)
    while r.sum() > H:
        r[np.argmax(r)] -= 1
    while r.sum() < H:
        r[np.argmin(r)] += 1
    return r


def _kv_id(x: np.ndarray, Wk: np.ndarray, cache: np.ndarray) -> np.ndarray:
    k = (x.reshape(B * P, DIM).astype(np.float32) @ Wk.astype(np.float32))
    k = k.reshape(B, P, HKV, D).transpose(0, 2, 1, 3)
    r = _ratios_np(k, cache.astype(np.float32))
    return np.searchsorted(np.cumsum(r), np.arange(H), side="right")


# ----------------------------------------------------- walrus wait splitting

def _split_wide_waits(nc, max_waits=1):
    """This toolchain's walrus allows only one sync-wait per instruction;
    move extra waits onto preceding NOPs on the same engine."""
    import bass_rust
    import concourse.mybir as mybir

    n = 0
    for f in nc.m.functions:
        for blk in f.blocks:
            out = []
            changed = False
            for ins in blk.instructions:
                si = ins.sync_info
                if si is not None and si.on_wait is not None and \
                        len(si.on_wait) > max_waits:
                    waits = list(si.on_wait)
                    keep = waits[-max_waits:]
                    extra = waits[:-max_waits]
                    for j in range(0, len(extra), max_waits):
                        n += 1
                        nop = mybir.InstNoOp(
                            name=f"waitsplit-{n}", ins=[], outs=[])
                        nop.engine = ins.engine
                        nop.sync_info = bass_rust.SyncInfo(
                            on_wait=extra[j:j + max_waits], on_update=[])
                        out.append(nop)
                    ins.sync_info = bass_rust.SyncInfo(
                        on_wait=keep, on_update=list(si.on_update or []))
                    changed = True
                out.append(ins)
            if changed:
                blk.instructions = out
    return n


# ------------------------------------------------------------ device program

def build_program(lmap, p_len: int = P, split_waits: bool = True):
    """Build the SPMD Bass/Tile program, specialized to the per-core local
    query-head -> kv-slot map `lmap` (tuple of HPC ints in [0, g))."""
    from contextlib import ExitStack

    import concourse.bass as bass
    import concourse.tile as tile
    from concourse import mybir

    F32 = mybir.dt.float32
    BF = mybir.dt.bfloat16
    EXP = mybir.ActivationFunctionType.Exp
    LN = mybir.ActivationFunctionType.Ln

    g = max(lmap) + 1         # distinct kv heads on this core
    GW = g * D                # k/v projection width
    NKT = (g + 1) // 2        # dedup'd kT row-tiles of 128

    NKC = p_len // 128            # key chunks
    NQT = max(p_len // 512, 1)    # q tiles
    QW = min(512, p_len)          # q tile width
    NRM = p_len // 128            # x row chunks
    RPT = NRM // NQT              # row chunks per q tile
    NDIN = DIM // 128             # contraction chunks for projections
    NPAIR = HPC // 2
    LAG = 4                       # PV chunks behind QK

    nc = bass.Bass("TRN2", target_bir_lowering=False, debug=False,
                   num_devices=NCORES)
    xt_d = nc.dram_tensor("xt", [DIM, p_len], BF, kind="ExternalInput").ap()
    wq_d = nc.dram_tensor("wq", [DIM, DPC], BF, kind="ExternalInput").ap()
    wk_d = nc.dram_tensor("wk", [DIM, GW], BF, kind="ExternalInput").ap()
    wv_d = nc.dram_tensor("wv", [DIM, GW], BF, kind="ExternalInput").ap()
    wp_d = nc.dram_tensor("wp", [DPC, DIM], BF, kind="ExternalInput").ap()
    out_d = nc.dram_tensor("out", [p_len, DIM], F32, kind="ExternalOutput").ap()

    with tile.TileContext(nc) as tc, ExitStack() as ctx:
        sbw = ctx.enter_context(tc.tile_pool(name="sbw", bufs=1))
        sbx = ctx.enter_context(tc.tile_pool(name="sbx", bufs=1))
        sbqk = ctx.enter_context(tc.tile_pool(name="sbqk", bufs=1))
        sbeg = ctx.enter_context(tc.tile_pool(name="sbeg", bufs=7))
        sbot = ctx.enter_context(tc.tile_pool(name="sbot", bufs=2))
        sbo = ctx.enter_context(tc.tile_pool(name="sbo", bufs=3))
        sbr = ctx.enter_context(tc.tile_pool(name="sbr", bufs=9))
        sbn = ctx.enter_context(tc.tile_pool(name="sbn", bufs=1))
        psb = ctx.enter_context(tc.tile_pool(name="psb", bufs=2, space="PSUM"))
        pssg = ctx.enter_context(tc.tile_pool(name="pssg", bufs=2, space="PSUM"))
        pspv = ctx.enter_context(tc.tile_pool(name="pspv", bufs=2, space="PSUM"))

        # ---- input loads -------------------------------------------------
        wq_sb = [sbw.tile([128, DPC], BF, tag=f"wq{i}", name=f"wq{i}") for i in range(NDIN)]
        wk_sb = [sbw.tile([128, GW], BF, tag=f"wk{i}", name=f"wk{i}") for i in range(NDIN)]
        wv_sb = [sbw.tile([128, GW], BF, tag=f"wv{i}", name=f"wv{i}") for i in range(NDIN)]
        xt_sb = [sbx.tile([128, p_len], BF, tag=f"xt{i}", name=f"xt{i}") for i in range(NDIN)]
        wp_sb = [sbw.tile([128, DIM], BF, tag=f"wp{i}", name=f"wp{i}") for i in range(DPC // 128)]
        # Order by first use so the serial HBM stream never gates the
        # pipeline start: xt+wk (kT chains), wq (qT), wv (v chains), wp
        # (projection, needed only ~2 units in).
        for i in range(NDIN):
            nc.sync.dma_start(xt_sb[i][:], xt_d[128 * i:128 * (i + 1), :])
            nc.sync.dma_start(wk_sb[i][:], wk_d[128 * i:128 * (i + 1), :])
            nc.sync.dma_start(wq_sb[i][:], wq_d[128 * i:128 * (i + 1), :])
        for i in range(NDIN):
            nc.sync.dma_start(wv_sb[i][:], wv_d[128 * i:128 * (i + 1), :])
        for i in range(DPC // 128):
            nc.sync.dma_start(wp_sb[i][:], wp_d[128 * i:128 * (i + 1), :])

        # ones rows at partitions {0,32,64,96} for the k=1 broadcast matmuls
        ones_sb = sbn.tile([128, 64], BF, tag="ones", name="ones")
        nc.vector.memset(ones_sb[:], 1.0)

        # normalization staging: sums rows / reciprocals for one t-tile live
        # at partitions {32p}; ScalarE cost is free-dim-bound, so one ln+exp
        # pair covers all 4 pairs of a t.
        sm_st = sbn.tile([128, 2 * QW], F32, tag="sm", name="sm")
        lg_st = sbn.tile([128, 2 * QW], F32, tag="lg", name="lg")
        rc_st = sbn.tile([128, 2 * QW], BF, tag="rc", name="rc")
        nc.vector.memset(sm_st[:], 1.0)

        # Touch Ln and Exp immediately so the ACT table set loads at kernel
        # start, long before the first real exp.
        warm = sbn.tile([1, 8], F32, tag="warm", name="warm")
        nc.vector.memset(warm[:], 1.0)
        nc.scalar.activation(warm[:], warm[:], LN)
        nc.scalar.activation(warm[:], warm[:], EXP)

        # ---- stage B: projection chain emitters ---------------------------
        # qT: [DPC, p_len]; ktd: dedup'd [g*64, p_len]; kt_qk: per-pair
        # [128, p_len] with the pair's two kv-head row blocks at partitions
        # 0:64 / 64:128 (so QK pairs row-pack in the PE array).
        qt_sb = [sbqk.tile([128, p_len], BF, tag=f"qt{m}", name=f"qt{m}") for m in range(DPC // 128)]
        ktd_sb = [sbqk.tile([128, p_len], BF, tag=f"ktd{m}", name=f"ktd{m}") for m in range(NKT)]
        ktq_sb = [sbqk.tile([128, p_len], BF, tag=f"ktq{m}", name=f"ktq{m}") for m in range(NPAIR)]
        vplus_sb = sbqk.tile([128, NKC * g * VW], BF, tag="vplus", name="vplus")
        vp3 = vplus_sb[:].rearrange("p (kc g w) -> p kc g w", kc=NKC, g=g)

        def vplus(kc, j):
            off = (kc * g + j) * VW
            return vplus_sb[:, off:off + VW]

        nc.vector.memset(vp3[:, :, :, D:VW], 1.0)

        def kt_dup(jt, t):
            """Fan dedup'd kT tile jt's fresh t-slice out to the QK tiles."""
            for pr in range(NPAIR):
                for half in (0, 1):
                    kv = lmap[2 * pr + half]
                    if kv // 2 == jt:
                        src = ktd_sb[jt][64 * (kv % 2):64 * (kv % 2) + 64,
                                         QW * t:QW * (t + 1)]
                        nc.vector.tensor_copy(
                            ktq_sb[pr][64 * half:64 * half + 64,
                                       QW * t:QW * (t + 1)], src)

        def qk_chain_gen(dst, w_sb, m, t, cols, post=None):
            ps = psb.tile([128, 512], F32, tag="psb", name="psb")
            for kd in range(NDIN):
                nc.tensor.matmul(
                    ps[0:cols, 0:QW], w_sb[kd][:, 128 * m:128 * m + cols],
                    xt_sb[kd][:, QW * t:QW * (t + 1)],
                    start=(kd == 0), stop=(kd == NDIN - 1))
                if kd % 2 == 1 and kd < NDIN - 1:
                    yield
            nc.vector.tensor_copy(dst[m][0:cols, QW * t:QW * (t + 1)],
                                  ps[0:cols, 0:QW])
            if post is not None:
                post()

        def v_chain_gen(rm):
            ps = psb.tile([128, 512], F32, tag="psb", name="psb")
            for kd in range(NDIN):
                nc.tensor.matmul(
                    ps[:, 0:GW], xt_sb[kd][:, 128 * rm:128 * (rm + 1)],
                    wv_sb[kd][:],
                    start=(kd == 0), stop=(kd == NDIN - 1))
                if kd % 2 == 1 and kd < NDIN - 1:
                    yield
            nc.vector.tensor_copy(
                vp3[:, rm, :, 0:D],
                ps[:, 0:GW].rearrange("p (g d) -> p g d", g=g))

        def proj_gen(t, rj, ot_tiles):
            o_sb = sbo.tile([128, DIM], F32, tag="osb", name="osb")
            for e2 in range(DIM // 512):
                ps = psb.tile([128, 512], F32, tag="psb", name="psb")
                for pair in range(NPAIR):
                    nc.tensor.matmul(
                        ps[:],
                        ot_tiles[pair][:, 128 * rj:128 * (rj + 1)],
                        wp_sb[pair][:, 512 * e2:512 * (e2 + 1)],
                        start=(pair == 0), stop=(pair == NPAIR - 1))
                    if pair == 1:
                        yield
                nc.vector.tensor_copy(o_sb[:, 512 * e2:512 * (e2 + 1)], ps[:])
                yield
            row0 = (t * RPT + rj) * 128
            nc.sync.dma_start(out_d[row0:row0 + 128, :], o_sb[:])

        import heapq

        total_chunks = NQT * NPAIR * NKC
        pump_q = []   # (deadline, seq, earliest, gen)
        pump_seq = [0]

        def add_gen(deadline, earliest, gen):
            pump_seq[0] += 1
            heapq.heappush(pump_q, (deadline, pump_seq[0], earliest, gen))

        # first QK tile of qt and the kt tiles it needs, eagerly (not pumped)
        def run_gen(gen):
            for _ in gen:
                pass

        for jt in range(NKT):
            run_gen(qk_chain_gen(ktd_sb, wk_sb, jt,
                                 0, min(128, GW - 128 * jt),
                                 post=(lambda jt=jt: kt_dup(jt, 0))))
        run_gen(qk_chain_gen(qt_sb, wq_sb, 0, 0, 128))

        for rm in range(NKC):
            add_gen(max(rm + 1, 0), 0, v_chain_gen(rm))
        for t in range(NQT):
            for m in range(DPC // 128):
                if m == 0 and t == 0:
                    continue
                dl = max((NQT * t + m) * NKC - 2, 0)
                add_gen(dl, 0, qk_chain_gen(qt_sb, wq_sb, m, t, 128))
            for jt in range(NKT):
                if t == 0:
                    continue
                dl = max((NQT * t) * NKC - 4, 0)
                add_gen(dl, 0, qk_chain_gen(
                    ktd_sb, wk_sb, jt, t, min(128, GW - 128 * jt),
                    post=(lambda jt=jt, t=t: kt_dup(jt, t))))

        pump_state = {"gen": None, "dl": 0}

        def pump(gg, budget=1):
            steps = 0
            while True:
                if pump_state["gen"] is None:
                    if not pump_q or pump_q[0][2] > gg:
                        return
                    dl, _, _, gen = heapq.heappop(pump_q)
                    pump_state["gen"] = gen
                    pump_state["dl"] = dl
                urgent = pump_state["dl"] <= gg + 2
                if steps >= budget and not urgent:
                    return
                try:
                    next(pump_state["gen"])
                    steps += 1
                except StopIteration:
                    pump_state["gen"] = None

        # ---- stage C + D: attention, normalization, projection ------------
        units = [(t, pair) for t in range(NQT) for pair in range(NPAIR)]

        class Unit:
            pass

        def start_unit(i):
            u = Unit()
            u.t, u.pair = units[i]
            u.h0, u.h1 = 2 * u.pair, 2 * u.pair + 1
            u.qt, u.kt = qt_sb[u.pair], ktq_sb[u.pair]
            u.pv0 = pspv.tile([128, QW], F32, tag="pv", name="pv")
            u.pv1 = pspv.tile([128, QW], F32, tag="pv", name="pv")
            u.egs = [None] * NKC
            u.fa_done = u.rc_done = False
            u.fb_done = 0
            return u

        def qk_exp(u, kc):
            sg = pssg.tile([128, 2 * QW], F32, tag="sg", name="sg")
            nc.tensor.matmul(
                sg[:, 0:QW], u.kt[0:64, 128 * kc:128 * (kc + 1)],
                u.qt[0:64, QW * u.t:QW * (u.t + 1)], start=True, stop=True)
            nc.tensor.matmul(
                sg[:, QW:2 * QW], u.kt[64:128, 128 * kc:128 * (kc + 1)],
                u.qt[64:128, QW * u.t:QW * (u.t + 1)], start=True, stop=True)
            eg = sbeg.tile([128, 2 * QW], BF, tag="eg", name="eg")
            nc.scalar.activation(eg[:], sg[:], EXP)
            u.egs[kc] = eg

        def pv_mm(u, kc):
            nc.tensor.matmul(
                u.pv0[0:VW, :], vplus(kc, lmap[u.h0]), u.egs[kc][:, 0:QW],
                start=(kc == 0), stop=(kc == NKC - 1))
            nc.tensor.matmul(
                u.pv1[0:VW, :], vplus(kc, lmap[u.h1]), u.egs[kc][:, QW:2 * QW],
                start=(kc == 0), stop=(kc == NKC - 1))
            u.egs[kc] = None

        def finalize_a(u):
            # copy unnormalized outT to SBUF (frees the pv psum banks) and
            # stage the raw sums rows at partition 32*pair of sm_st.
            u.s0 = sbr.tile([64, QW], F32, tag="s", name="s")
            u.s1 = sbr.tile([64, QW], F32, tag="s", name="s")
            nc.vector.tensor_copy(u.s0[:], u.pv0[0:D, :])
            nc.vector.tensor_copy(u.s1[:], u.pv1[0:D, :])
            pp = 32 * u.pair
            nc.vector.tensor_copy(sm_st[pp:pp + 1, 0:QW], u.pv0[D:VW, :])
            nc.vector.tensor_copy(sm_st[pp:pp + 1, QW:2 * QW], u.pv1[D:VW, :])
            u.fa_done = True

        def recip_t():
            # reciprocal of all 8 sums rows of this t as exp(-ln s):
            # both functions live in one ACT table set, and the batched
            # [128, 2QW] layout makes the two activations cost the same as
            # a single pair's would.
            nc.scalar.activation(lg_st[:], sm_st[:], LN)
            nc.scalar.activation(rc_st[:], lg_st[:], EXP, scale=-1.0)

        def finalize_b(u, ot_tiles):
            # broadcast reciprocals across partitions (k=1 matmuls, col-tiled
            # into one psum tile) and scale on VectorE.
            pp = 32 * u.pair
            rb = psb.tile([128, QW], F32, tag="psb", name="psb")
            nc.tensor.matmul(rb[0:64, :], ones_sb[pp:pp + 1, :],
                             rc_st[pp:pp + 1, 0:QW], start=True, stop=True,
                             tile_position=(pp, 0))
            nc.tensor.matmul(rb[64:128, :], ones_sb[pp:pp + 1, :],
                             rc_st[pp:pp + 1, QW:2 * QW], start=True,
                             stop=True, tile_position=(pp, 64))
            ot = sbot.tile([128, QW], BF, tag=f"ot{u.pair}", name=f"ot{u.pair}")
            nc.vector.tensor_mul(ot[0:64, :], u.s0[:], rb[0:64, :])
            nc.vector.tensor_mul(ot[64:128, :], u.s1[:], rb[64:128, :])
            ot_tiles[u.pair] = ot

        ot_by_t = {t: [None] * NPAIR for t in range(NQT)}

        def step_prev(u, kc, gg):
            # deferred post-processing of the previous unit(s), spread over
            # this unit's chunk stream
            if kc >= LAG and not u.fa_done:
                finalize_a(u)
            elif u.pair == NPAIR - 1 and u.fa_done:
                if kc >= LAG + 2 and not u.rc_done:
                    recip_t()
                    u.rc_done = True
                elif u.rc_done and u.fb_done < NPAIR and kc >= LAG + 3 + u.fb_done:
                    finalize_b(u.t_units[u.fb_done], ot_by_t[u.t])
                    u.fb_done += 1
                    if u.fb_done == NPAIR:
                        for rj in range(RPT):
                            add_gen(gg + 8 + rj, gg + 1,
                                    proj_gen(u.t, rj, ot_by_t[u.t]))

        def drain_prev(u, gg):
            if not u.fa_done:
                finalize_a(u)
            if u.pair == NPAIR - 1:
                if not u.rc_done:
                    recip_t()
                    u.rc_done = True
                while u.fb_done < NPAIR:
                    finalize_b(u.t_units[u.fb_done], ot_by_t[u.t])
                    u.fb_done += 1
                for rj in range(RPT):
                    add_gen(gg + 8 + rj, 0, proj_gen(u.t, rj, ot_by_t[u.t]))

        prev = None
        t_units = []
        cur = start_unit(0)
        for i in range(len(units)):
            t_units.append(cur)
            if cur.pair == NPAIR - 1:
                cur.t_units = t_units
                t_units = []
            for kc in range(NKC):
                gg = i * NKC + kc
                qk_exp(cur, kc)
                gk = kc - LAG
                if gk >= 0:
                    pv_mm(cur, gk)
                elif prev is not None:
                    pv_mm(prev, NKC + gk)
                pump(gg)
                if prev is not None:
                    step_prev(prev, kc, gg)
            if prev is not None and (not prev.fa_done or
                                     (prev.pair == NPAIR - 1 and
                                      prev.fb_done < NPAIR)):
                drain_prev(prev, i * NKC + NKC - 1)
            prev, cur = cur, (start_unit(i + 1) if i + 1 < len(units) else None)
        for gk in range(NKC - LAG, NKC):
            pv_mm(prev, gk)
            pump(total_chunks)
        drain_prev(prev, total_chunks)
        gg = total_chunks
        while pump_q or pump_state["gen"] is not None:
            pump(gg, budget=100)
            gg += 1

    if split_waits:
        _split_wide_waits(nc, max_waits=1)
    return nc


_PROGRAMS = {}


def _get_program(lmap):
    if lmap not in _PROGRAMS:
        _PROGRAMS[lmap] = build_program(lmap, P)
    return _PROGRAMS[lmap]


# ------------------------------------------------------------------- kernel

def make_in_maps(x, Wq, Wk, Wv, Wp, bp, cache):
    x = np.asarray(x, np.float32)
    Wq = np.asarray(Wq, np.float32)
    Wk = np.asarray(Wk, np.float32)
    Wv = np.asarray(Wv, np.float32)
    Wp = np.asarray(Wp, np.float32)
    kv_id = _kv_id(x, Wk, np.asarray(cache, np.float32))

    # per-core local maps; specialize if identical across cores
    maps, dists = [], []
    for c in range(NCORES):
        half = c % 2
        ids = [int(kv_id[h]) for h in range(half * HPC, (half + 1) * HPC)]
        uniq = sorted(set(ids))
        maps.append(tuple(uniq.index(i) for i in ids))
        dists.append(uniq)
    if all(m == maps[0] for m in maps):
        lmap = maps[0]
    else:
        lmap = tuple(range(HPC))
        dists = [[int(kv_id[h]) for h in range((c % 2) * HPC,
                                               (c % 2 + 1) * HPC)]
                 for c in range(NCORES)]

    scale = 1.0 / np.sqrt(D)
    in_maps = []
    xt_b = [np.ascontiguousarray(x[b].T).astype(BF16) for b in range(B)]
    for c in range(NCORES):
        b, half = divmod(c, 2)
        wk_c = np.concatenate(
            [Wk[:, d * D:(d + 1) * D] for d in dists[c]], axis=1)
        wv_c = np.concatenate(
            [Wv[:, d * D:(d + 1) * D] for d in dists[c]], axis=1)
        in_maps.append({
            "xt": xt_b[b],
            "wq": (Wq[:, half * DPC:(half + 1) * DPC] * scale).astype(BF16),
            "wk": np.ascontiguousarray(wk_c).astype(BF16),
            "wv": np.ascontiguousarray(wv_c).astype(BF16),
            "wp": np.ascontiguousarray(
                Wp[half * DPC:(half + 1) * DPC, :]).astype(BF16),
        })
    return in_maps, lmap


_WARMED = set()


def kernel(x, Wq, Wk, Wv, Wp, bp, cache, _trace=False):
    from concourse.bass_utils import run_bass_kernel_spmd

    in_maps, lmap = make_in_maps(x, Wq, Wk, Wv, Wp, bp, cache)
    nc = _get_program(lmap)
    if lmap not in _WARMED:
        # First execution on a cold NEFF has been observed racing the ACT
        # table load; run once and discard.
        run_bass_kernel_spmd(nc, in_maps, core_ids=list(range(NCORES)),
                             trace=False)
        _WARMED.add(lmap)
    res = run_bass_kernel_spmd(nc, in_maps, core_ids=list(range(NCORES)),
                               trace=_trace)
    bp32 = np.asarray(bp, np.float32)
    out = np.empty((B, P, DIM), np.float32)
    for b in range(B):
        out[b] = res.results[2 * b]["out"] + res.results[2 * b + 1]["out"] + bp32
    if _trace:
        kernel.last_exec_time_ns = res.exec_time_ns
    return out


# revision 12
# speedup vs baseline: 1.0145x; 1.0117x over previous
"""Dynamic GQA attention (nn_DGQA) on 8 Trainium2 NeuronCores.

Strategy (v2)
-------------
The dynamic query-head -> kv-head assignment is computed on the host and the
device program is SPECIALIZED to the resulting per-core local grouping
pattern (cached per pattern; identical across cores for the uniform case,
with an identity-map fallback that reproduces the v1 per-head-duplicated
layout).  With g distinct kv heads per core the k/v projections are computed
at width g*64 instead of 8*64, and the kT rows needed by the row-packed QK
pairs are fanned out by cheap DVE copies.

Sharding: core c handles batch b = c//2 and half of the query heads
(half = c%2).  Each core computes a partial output projection; the host sums
the two partials per batch and adds the bias.

Device kernel (per core, all matmuls bf16, fp32 accumulation):
  xt [1024,2048] = x[b].T
  qT = wq.T @ xt            (wq pre-scaled by D^-0.5 on host)
  ktd = wk.T @ xt           (dedup'd: g kv heads)  -> DVE fan-out to kt_qk
  v  = xt.T @ wv            (dedup'd, strided into "vplus" with ones col)
  per head pair (row-tiled QK at K=64, 2 heads concurrent):
    scoresT[key, qp] in PSUM -> exp on ScalarE -> bf16 SBUF
    PV: outT[d, qp] (+ sums row via ones column), v stationary
  softmax normalization: sums rows staged at partitions {0,32,64,96} of a
  [128, 2*QW] tile; ONE ln + ONE exp per t-tile covers all 4 pairs
  (ScalarE time is free-dim-bound, so batching partitions is ~4x cheaper);
  reciprocals broadcast with col-tiled k=1 matmuls, multiplied on VectorE
  proj: out[row, :] partial = outT.T @ wp
"""

import numpy as np
import ml_dtypes

B, P, DIM, H, HKV = 4, 2048, 1024, 16, 8
D = DIM // H          # 64
NCORES = 8
HPC = H // 2          # query heads per core = 8
DPC = HPC * D         # query head-dims per core = 512
VW = D + 1            # v columns per head incl. ones column = 65

BF16 = ml_dtypes.bfloat16


# ----------------------------------------------------------------- host math

def _ratios_np(k_bhpd: np.ndarray, cache: np.ndarray) -> np.ndarray:
    """Numpy replica of the reference's _ratios (fp32, round-half-even)."""
    mags = np.sqrt((k_bhpd * k_bhpd).sum(axis=(2, 3))).sum(axis=0)
    diff = np.abs(cache - mags)
    r = np.round(diff / diff.sum() * H).astype(np.int64)
    while r.sum() > H:
        r[np.argmax(r)] -= 1
    while r.sum() < H:
        r[np.argmin(r)] += 1
    return r


def _kv_id(x: np.ndarray, Wk: np.ndarray, cache: np.ndarray) -> np.ndarray:
    k = (x.reshape(B * P, DIM).astype(np.float32) @ Wk.astype(np.float32))
    k = k.reshape(B, P, HKV, D).transpose(0, 2, 1, 3)
    r = _ratios_np(k, cache.astype(np.float32))
    return np.searchsorted(np.cumsum(r), np.arange(H), side="right")


# ----------------------------------------------------- walrus wait splitting

def _split_wide_waits(nc, max_waits=1):
    """This toolchain's walrus allows only one sync-wait per instruction;
    move extra waits onto preceding NOPs on the same engine."""
    import bass_rust
    import concourse.mybir as mybir

    n = 0
    for f in nc.m.functions:
        for blk in f.blocks:
            out = []
            changed = False
            for ins in blk.instructions:
                si = ins.sync_info
                if si is not None and si.on_wait is not None and \
                        len(si.on_wait) > max_waits:
                    waits = list(si.on_wait)
                    keep = waits[-max_waits:]
                    extra = waits[:-max_waits]
                    for j in range(0, len(extra), max_waits):
                        n += 1
                        nop = mybir.InstNoOp(
                            name=f"waitsplit-{n}", ins=[], outs=[])
                        nop.engine = ins.engine
                        nop.sync_info = bass_rust.SyncInfo(
                            on_wait=extra[j:j + max_waits], on_update=[])
                        out.append(nop)
                    ins.sync_info = bass_rust.SyncInfo(
                        on_wait=keep, on_update=list(si.on_update or []))
                    changed = True
                out.append(ins)
            if changed:
                blk.instructions = out
    return n


# ------------------------------------------------------------ device program

def build_program(lmap, p_len: int = P, split_waits: bool = True):
    """Build the SPMD Bass/Tile program, specialized to the per-core local
    query-head -> kv-slot map `lmap` (tuple of HPC ints in [0, g))."""
    from contextlib import ExitStack

    import concourse.bass as bass
    import concourse.tile as tile
    from concourse import mybir

    F32 = mybir.dt.float32
    BF = mybir.dt.bfloat16
    EXP = mybir.ActivationFunctionType.Exp
    LN = mybir.ActivationFunctionType.Ln

    g = max(lmap) + 1         # distinct kv heads on this core
    GW = g * D                # k/v projection width
    NKT = (g + 1) // 2        # dedup'd kT row-tiles of 128

    NKC = p_len // 128            # key chunks
    NQT = max(p_len // 512, 1)    # q tiles
    QW = min(512, p_len)          # q tile width
    NRM = p_len // 128            # x row chunks
    RPT = NRM // NQT              # row chunks per q tile
    NDIN = DIM // 128             # contraction chunks for projections
    NPAIR = HPC // 2
    LAG = 4                       # PV chunks behind QK

    nc = bass.Bass("TRN2", target_bir_lowering=False, debug=False,
                   num_devices=NCORES)
    xt_d = nc.dram_tensor("xt", [DIM, p_len], BF, kind="ExternalInput").ap()
    wq_d = nc.dram_tensor("wq", [DIM, DPC], BF, kind="ExternalInput").ap()
    wk_d = nc.dram_tensor("wk", [DIM, GW], BF, kind="ExternalInput").ap()
    wv_d = nc.dram_tensor("wv", [DIM, GW], BF, kind="ExternalInput").ap()
    wp_d = nc.dram_tensor("wp", [DPC, DIM], BF, kind="ExternalInput").ap()
    out_d = nc.dram_tensor("out", [p_len, DIM], F32, kind="ExternalOutput").ap()

    with tile.TileContext(nc) as tc, ExitStack() as ctx:
        sbw = ctx.enter_context(tc.tile_pool(name="sbw", bufs=1))
        sbx = ctx.enter_context(tc.tile_pool(name="sbx", bufs=1))
        sbqk = ctx.enter_context(tc.tile_pool(name="sbqk", bufs=1))
        sbeg = ctx.enter_context(tc.tile_pool(name="sbeg", bufs=7))
        sbot = ctx.enter_context(tc.tile_pool(name="sbot", bufs=2))
        sbo = ctx.enter_context(tc.tile_pool(name="sbo", bufs=3))
        sbr = ctx.enter_context(tc.tile_pool(name="sbr", bufs=9))
        sbn = ctx.enter_context(tc.tile_pool(name="sbn", bufs=1))
        psb = ctx.enter_context(tc.tile_pool(name="psb", bufs=2, space="PSUM"))
        pssg = ctx.enter_context(tc.tile_pool(name="pssg", bufs=2, space="PSUM"))
        pspv = ctx.enter_context(tc.tile_pool(name="pspv", bufs=2, space="PSUM"))

        # ---- input loads -------------------------------------------------
        # Weights load as ONE batched DMA each (fewer ring slots, one
        # completion); xt stays 8 per-piece DMAs so the projection chains
        # can consume contraction chunks as they land.  Issue order = order
        # of first use: wk/wq gate the first QK, wv the first PV, wp only
        # the projection ~2 units in.
        def batched_w(name, dram, width, n):
            t = sbw.tile([128, n * width], BF, tag=name, name=name)
            nc.sync.dma_start(
                t[:].rearrange("p (i c) -> p i c", i=n),
                dram[:, :].rearrange("(i p) c -> p i c", p=128))
            return [t[:, width * i:width * (i + 1)] for i in range(n)]

        wk_sb = batched_w("wk", wk_d, GW, NDIN)
        wq_sb = batched_w("wq", wq_d, DPC, NDIN)
        xt_sb = [sbx.tile([128, p_len], BF, tag=f"xt{i}", name=f"xt{i}") for i in range(NDIN)]
        for i in range(NDIN):
            nc.sync.dma_start(xt_sb[i][:], xt_d[128 * i:128 * (i + 1), :])
        wv_sb = batched_w("wv", wv_d, GW, NDIN)
        wp_sb = batched_w("wp", wp_d, DIM, DPC // 128)

        # ones rows at partitions {0,32,64,96} for the k=1 broadcast matmuls
        ones_sb = sbn.tile([128, 64], BF, tag="ones", name="ones")
        nc.vector.memset(ones_sb[:], 1.0)

        # normalization staging: sums rows / reciprocals for one t-tile live
        # at partitions {32p}; ScalarE cost is free-dim-bound, so one ln+exp
        # pair covers all 4 pairs of a t.
        sm_st = sbn.tile([128, 2 * QW], F32, tag="sm", name="sm")
        lg_st = sbn.tile([128, 2 * QW], F32, tag="lg", name="lg")
        rc_st = sbn.tile([128, 2 * QW], BF, tag="rc", name="rc")
        nc.vector.memset(sm_st[:], 1.0)

        # Touch Ln and Exp immediately so the ACT table set loads at kernel
        # start, long before the first real exp.
        warm = sbn.tile([1, 8], F32, tag="warm", name="warm")
        nc.vector.memset(warm[:], 1.0)
        nc.scalar.activation(warm[:], warm[:], LN)
        nc.scalar.activation(warm[:], warm[:], EXP)

        # ---- stage B: projection chain emitters ---------------------------
        # qT: [DPC, p_len]; ktd: dedup'd [g*64, p_len]; kt_qk: per-pair
        # [128, p_len] with the pair's two kv-head row blocks at partitions
        # 0:64 / 64:128 (so QK pairs row-pack in the PE array).
        qt_sb = [sbqk.tile([128, p_len], BF, tag=f"qt{m}", name=f"qt{m}") for m in range(DPC // 128)]
        ktd_sb = [sbqk.tile([128, p_len], BF, tag=f"ktd{m}", name=f"ktd{m}") for m in range(NKT)]
        ktq_sb = [sbqk.tile([128, p_len], BF, tag=f"ktq{m}", name=f"ktq{m}") for m in range(NPAIR)]
        vplus_sb = sbqk.tile([128, NKC * g * VW], BF, tag="vplus", name="vplus")
        vp3 = vplus_sb[:].rearrange("p (kc g w) -> p kc g w", kc=NKC, g=g)

        def vplus(kc, j):
            off = (kc * g + j) * VW
            return vplus_sb[:, off:off + VW]

        nc.vector.memset(vp3[:, :, :, D:VW], 1.0)

        def kt_dup(jt, t):
            """Fan dedup'd kT tile jt's fresh t-slice out to the QK tiles."""
            for pr in range(NPAIR):
                for half in (0, 1):
                    kv = lmap[2 * pr + half]
                    if kv // 2 == jt:
                        src = ktd_sb[jt][64 * (kv % 2):64 * (kv % 2) + 64,
                                         QW * t:QW * (t + 1)]
                        nc.vector.tensor_copy(
                            ktq_sb[pr][64 * half:64 * half + 64,
                                       QW * t:QW * (t + 1)], src)

        def qk_chain_gen(dst, w_sb, m, t, cols, post=None, pool=None):
            ps = (pool or psb).tile([128, 512], F32,
                                    tag="sg" if pool else "psb", name="psb")
            for kd in range(NDIN):
                nc.tensor.matmul(
                    ps[0:cols, 0:QW], w_sb[kd][:, 128 * m:128 * m + cols],
                    xt_sb[kd][:, QW * t:QW * (t + 1)],
                    start=(kd == 0), stop=(kd == NDIN - 1))
                if kd % 2 == 1 and kd < NDIN - 1:
                    yield
            nc.vector.tensor_copy(dst[m][0:cols, QW * t:QW * (t + 1)],
                                  ps[0:cols, 0:QW])
            if post is not None:
                post()

        def v_chain_gen(rm):
            ps = psb.tile([128, 512], F32, tag="psb", name="psb")
            for kd in range(NDIN):
                nc.tensor.matmul(
                    ps[:, 0:GW], xt_sb[kd][:, 128 * rm:128 * (rm + 1)],
                    wv_sb[kd][:],
                    start=(kd == 0), stop=(kd == NDIN - 1))
                if kd % 2 == 1 and kd < NDIN - 1:
                    yield
            nc.vector.tensor_copy(
                vp3[:, rm, :, 0:D],
                ps[:, 0:GW].rearrange("p (g d) -> p g d", g=g))

        def proj_gen(t, rj, ot_tiles):
            o_sb = sbo.tile([128, DIM], F32, tag="osb", name="osb")
            for e2 in range(DIM // 512):
                ps = psb.tile([128, 512], F32, tag="psb", name="psb")
                for pair in range(NPAIR):
                    nc.tensor.matmul(
                        ps[:],
                        ot_tiles[pair][:, 128 * rj:128 * (rj + 1)],
                        wp_sb[pair][:, 512 * e2:512 * (e2 + 1)],
                        start=(pair == 0), stop=(pair == NPAIR - 1))
                    if pair == 1:
                        yield
                nc.vector.tensor_copy(o_sb[:, 512 * e2:512 * (e2 + 1)], ps[:])
                yield
            row0 = (t * RPT + rj) * 128
            nc.sync.dma_start(out_d[row0:row0 + 128, :], o_sb[:])

        import heapq

        total_chunks = NQT * NPAIR * NKC
        pump_q = []   # (deadline, seq, earliest, gen)
        pump_seq = [0]

        def add_gen(deadline, earliest, gen):
            pump_seq[0] += 1
            heapq.heappush(pump_q, (deadline, pump_seq[0], earliest, gen))

        # first QK tile of qt and the kt tiles it needs, eagerly (not pumped)
        def run_gen(gen):
            for _ in gen:
                pass

        # Interleave the eager chains' matmuls so all of them advance as the
        # xt pieces land (ktd chains on the 2 psb slots, qt on a free sg
        # slot — attention hasn't started yet).
        eager = [qk_chain_gen(ktd_sb, wk_sb, jt, 0, min(128, GW - 128 * jt),
                              post=(lambda jt=jt: kt_dup(jt, 0)))
                 for jt in range(NKT)]
        eager.append(qk_chain_gen(qt_sb, wq_sb, 0, 0, 128, pool=pssg))
        while eager:
            for gen in list(eager):
                try:
                    next(gen)
                except StopIteration:
                    eager.remove(gen)

        for rm in range(NKC):
            add_gen(max(rm + 1, 0), 0, v_chain_gen(rm))
        for t in range(NQT):
            for m in range(DPC // 128):
                if m == 0 and t == 0:
                    continue
                dl = max((NQT * t + m) * NKC - 2, 0)
                add_gen(dl, 0, qk_chain_gen(qt_sb, wq_sb, m, t, 128))
            for jt in range(NKT):
                if t == 0:
                    continue
                dl = max((NQT * t) * NKC - 4, 0)
                add_gen(dl, 0, qk_chain_gen(
                    ktd_sb, wk_sb, jt, t, min(128, GW - 128 * jt),
                    post=(lambda jt=jt, t=t: kt_dup(jt, t))))

        pump_state = {"gen": None, "dl": 0}

        def pump(gg, budget=1):
            steps = 0
            while True:
                if pump_state["gen"] is None:
                    if not pump_q or pump_q[0][2] > gg:
                        return
                    dl, _, _, gen = heapq.heappop(pump_q)
                    pump_state["gen"] = gen
                    pump_state["dl"] = dl
                urgent = pump_state["dl"] <= gg + 2
                if steps >= budget and not urgent:
                    return
                try:
                    next(pump_state["gen"])
                    steps += 1
                except StopIteration:
                    pump_state["gen"] = None

        # ---- stage C + D: attention, normalization, projection ------------
        units = [(t, pair) for t in range(NQT) for pair in range(NPAIR)]

        class Unit:
            pass

        def start_unit(i):
            u = Unit()
            u.t, u.pair = units[i]
            u.h0, u.h1 = 2 * u.pair, 2 * u.pair + 1
            u.qt, u.kt = qt_sb[u.pair], ktq_sb[u.pair]
            u.pv0 = pspv.tile([128, QW], F32, tag="pv", name="pv")
            u.pv1 = pspv.tile([128, QW], F32, tag="pv", name="pv")
            u.egs = [None] * NKC
            u.fa_done = u.rc_done = False
            u.fb_done = 0
            return u

        def qk_exp(u, kc):
            sg = pssg.tile([128, 2 * QW], F32, tag="sg", name="sg")
            nc.tensor.matmul(
                sg[:, 0:QW], u.kt[0:64, 128 * kc:128 * (kc + 1)],
                u.qt[0:64, QW * u.t:QW * (u.t + 1)], start=True, stop=True)
            nc.tensor.matmul(
                sg[:, QW:2 * QW], u.kt[64:128, 128 * kc:128 * (kc + 1)],
                u.qt[64:128, QW * u.t:QW * (u.t + 1)], start=True, stop=True)
            eg = sbeg.tile([128, 2 * QW], BF, tag="eg", name="eg")
            nc.scalar.activation(eg[:], sg[:], EXP)
            u.egs[kc] = eg

        def pv_mm(u, kc):
            nc.tensor.matmul(
                u.pv0[0:VW, :], vplus(kc, lmap[u.h0]), u.egs[kc][:, 0:QW],
                start=(kc == 0), stop=(kc == NKC - 1))
            nc.tensor.matmul(
                u.pv1[0:VW, :], vplus(kc, lmap[u.h1]), u.egs[kc][:, QW:2 * QW],
                start=(kc == 0), stop=(kc == NKC - 1))
            u.egs[kc] = None

        def finalize_a(u):
            # copy unnormalized outT to SBUF (frees the pv psum banks) and
            # stage the raw sums rows at partition 32*pair of sm_st.
            u.s0 = sbr.tile([64, QW], F32, tag="s", name="s")
            u.s1 = sbr.tile([64, QW], F32, tag="s", name="s")
            nc.vector.tensor_copy(u.s0[:], u.pv0[0:D, :])
            nc.vector.tensor_copy(u.s1[:], u.pv1[0:D, :])
            pp = 32 * u.pair
            nc.vector.tensor_copy(sm_st[pp:pp + 1, 0:QW], u.pv0[D:VW, :])
            nc.vector.tensor_copy(sm_st[pp:pp + 1, QW:2 * QW], u.pv1[D:VW, :])
            u.fa_done = True

        def recip_t():
            # reciprocal of all 8 sums rows of this t as exp(-ln s):
            # both functions live in one ACT table set, and the batched
            # [128, 2QW] layout makes the two activations cost the same as
            # a single pair's would.
            nc.scalar.activation(lg_st[:], sm_st[:], LN)
            nc.scalar.activation(rc_st[:], lg_st[:], EXP, scale=-1.0)

        def finalize_b(u, ot_tiles):
            # broadcast reciprocals across partitions (k=1 matmuls, col-tiled
            # into one psum tile) and scale on VectorE.
            pp = 32 * u.pair
            rb = psb.tile([128, QW], F32, tag="psb", name="psb")
            nc.tensor.matmul(rb[0:64, :], ones_sb[pp:pp + 1, :],
                             rc_st[pp:pp + 1, 0:QW], start=True, stop=True,
                             tile_position=(pp, 0))
            nc.tensor.matmul(rb[64:128, :], ones_sb[pp:pp + 1, :],
                             rc_st[pp:pp + 1, QW:2 * QW], start=True,
                             stop=True, tile_position=(pp, 64))
            ot = sbot.tile([128, QW], BF, tag=f"ot{u.pair}", name=f"ot{u.pair}")
            nc.vector.tensor_mul(ot[0:64, :], u.s0[:], rb[0:64, :])
            nc.vector.tensor_mul(ot[64:128, :], u.s1[:], rb[64:128, :])
            ot_tiles[u.pair] = ot

        ot_by_t = {t: [None] * NPAIR for t in range(NQT)}

        def step_prev(u, kc, gg):
            # deferred post-processing of the previous unit(s), spread over
            # this unit's chunk stream
            if kc >= LAG and not u.fa_done:
                finalize_a(u)
            elif u.pair == NPAIR - 1 and u.fa_done:
                if kc >= LAG + 2 and not u.rc_done:
                    recip_t()
                    u.rc_done = True
                elif u.rc_done and u.fb_done < NPAIR and kc >= LAG + 3 + u.fb_done:
                    finalize_b(u.t_units[u.fb_done], ot_by_t[u.t])
                    u.fb_done += 1
                    if u.fb_done == NPAIR:
                        for rj in range(RPT):
                            add_gen(gg + 8 + rj, gg + 1,
                                    proj_gen(u.t, rj, ot_by_t[u.t]))

        def drain_prev(u, gg):
            if not u.fa_done:
                finalize_a(u)
            if u.pair == NPAIR - 1:
                if not u.rc_done:
                    recip_t()
                    u.rc_done = True
                while u.fb_done < NPAIR:
                    finalize_b(u.t_units[u.fb_done], ot_by_t[u.t])
                    u.fb_done += 1
                for rj in range(RPT):
                    add_gen(gg + 8 + rj, 0, proj_gen(u.t, rj, ot_by_t[u.t]))

        prev = None
        t_units = []
        cur = start_unit(0)
        for i in range(len(units)):
            t_units.append(cur)
            if cur.pair == NPAIR - 1:
                cur.t_units = t_units
                t_units = []
            for kc in range(NKC):
                gg = i * NKC + kc
                qk_exp(cur, kc)
                gk = kc - LAG
                if gk >= 0:
                    pv_mm(cur, gk)
                elif prev is not None:
                    pv_mm(prev, NKC + gk)
                pump(gg)
                if prev is not None:
                    step_prev(prev, kc, gg)
            if prev is not None and (not prev.fa_done or
                                     (prev.pair == NPAIR - 1 and
                                      prev.fb_done < NPAIR)):
                drain_prev(prev, i * NKC + NKC - 1)
            prev, cur = cur, (start_unit(i + 1) if i + 1 < len(units) else None)
        for gk in range(NKC - LAG, NKC):
            pv_mm(prev, gk)
            pump(total_chunks)
        drain_prev(prev, total_chunks)
        gg = total_chunks
        while pump_q or pump_state["gen"] is not None:
            pump(gg, budget=100)
            gg += 1

    if split_waits:
        _split_wide_waits(nc, max_waits=1)
    return nc


_PROGRAMS = {}


def _get_program(lmap):
    if lmap not in _PROGRAMS:
        _PROGRAMS[lmap] = build_program(lmap, P)
    return _PROGRAMS[lmap]


# ------------------------------------------------------------------- kernel

def make_in_maps(x, Wq, Wk, Wv, Wp, bp, cache):
    x = np.asarray(x, np.float32)
    Wq = np.asarray(Wq, np.float32)
    Wk = np.asarray(Wk, np.float32)
    Wv = np.asarray(Wv, np.float32)
    Wp = np.asarray(Wp, np.float32)
    kv_id = _kv_id(x, Wk, np.asarray(cache, np.float32))

    # per-core local maps; specialize if identical across cores
    maps, dists = [], []
    for c in range(NCORES):
        half = c % 2
        ids = [int(kv_id[h]) for h in range(half * HPC, (half + 1) * HPC)]
        uniq = sorted(set(ids))
        maps.append(tuple(uniq.index(i) for i in ids))
        dists.append(uniq)
    if all(m == maps[0] for m in maps):
        lmap = maps[0]
    else:
        lmap = tuple(range(HPC))
        dists = [[int(kv_id[h]) for h in range((c % 2) * HPC,
                                               (c % 2 + 1) * HPC)]
                 for c in range(NCORES)]

    scale = 1.0 / np.sqrt(D)
    in_maps = []
    xt_b = [np.ascontiguousarray(x[b].T).astype(BF16) for b in range(B)]
    for c in range(NCORES):
        b, half = divmod(c, 2)
        wk_c = np.concatenate(
            [Wk[:, d * D:(d + 1) * D] for d in dists[c]], axis=1)
        wv_c = np.concatenate(
            [Wv[:, d * D:(d + 1) * D] for d in dists[c]], axis=1)
        in_maps.append({
            "xt": xt_b[b],
            "wq": (Wq[:, half * DPC:(half + 1) * DPC] * scale).astype(BF16),
            "wk": np.ascontiguousarray(wk_c).astype(BF16),
            "wv": np.ascontiguousarray(wv_c).astype(BF16),
            "wp": np.ascontiguousarray(
                Wp[half * DPC:(half + 1) * DPC, :]).astype(BF16),
        })
    return in_maps, lmap


_WARMED = set()


def kernel(x, Wq, Wk, Wv, Wp, bp, cache, _trace=False):
    from concourse.bass_utils import run_bass_kernel_spmd

    in_maps, lmap = make_in_maps(x, Wq, Wk, Wv, Wp, bp, cache)
    nc = _get_program(lmap)
    if lmap not in _WARMED:
        # First execution on a cold NEFF has been observed racing the ACT
        # table load; run once and discard.
        run_bass_kernel_spmd(nc, in_maps, core_ids=list(range(NCORES)),
                             trace=False)
        _WARMED.add(lmap)
    res = run_bass_kernel_spmd(nc, in_maps, core_ids=list(range(NCORES)),
                               trace=_trace)
    bp32 = np.asarray(bp, np.float32)
    out = np.empty((B, P, DIM), np.float32)
    for b in range(B):
        out[b] = res.results[2 * b]["out"] + res.results[2 * b + 1]["out"] + bp32
    if _trace:
        kernel.last_exec_time_ns = res.exec_time_ns
    return out
